# revision 10
# baseline (speedup 1.0000x reference)
"""Trainium2 Bass kernel for nn_LAMME (conv3x3 + LAM temporal attention + ME gate).

Data-parallel over 8 NeuronCores: each core processes one clip of t=8 frames
(c=256, h=w=56). Single fused kernel per core.

Key structure (v2): the LAM/ME gates depend only on the spatial means of the
conv output, which are computed EXACTLY from the input via edge-corrected
strip sums (sum_hw(conv(x)) = sum_taps w . T where T[ci,dy,dx] = S - excluded
row - excluded col + corner).  A cheap prepass (DVE strip reduces + ACT
full-frame accum) therefore yields all gates ~100us into the kernel, letting
the per-frame temporal-conv + gate phase (phase 2) run fully overlapped with
the conv matmul stream instead of as a 107us serial tail.
"""
import sys
for p in ('/opt/trn_rl_repo',):
    if p not in sys.path:
        sys.path.insert(0, p)

import numpy as np
import ml_dtypes

import concourse.bacc as bacc
import concourse.mybir as mybir
import concourse.tile as tile
from concourse.bass_utils import run_bass_kernel_spmd

F32 = mybir.dt.float32
BF16 = mybir.dt.bfloat16
AF = mybir.ActivationFunctionType
OP = mybir.AluOpType

T = 8          # frames per clip (= clips per core after sharding)
NCORES = 8
NSP = 7        # spatial tiles per frame (56 rows / 8)
TW = 448       # 8 rows x 56 cols per tile
HP = 58        # padded spatial width
PADSZ = HP * HP  # 3364

_CACHE = {}


def _build(me):
    nc = bacc.Bacc("TRN2", target_bir_lowering=False, debug=False)

    x_d = nc.dram_tensor("x", [T, 2, 128, PADSZ], BF16, kind="ExternalInput")
    wt_d = nc.dram_tensor("wt", [128, 36, 128], BF16, kind="ExternalInput")
    lamw_d = nc.dram_tensor("lamw", [4, 128, 128], BF16, kind="ExternalInput")
    w1t_d = nc.dram_tensor("w1t", [8, 16], BF16, kind="ExternalInput")
    w2t_d = nc.dram_tensor("w2t", [16, 3], BF16, kind="ExternalInput")
    bns_d = nc.dram_tensor("bns", [16, 1], F32, kind="ExternalInput")
    bnsh_d = nc.dram_tensor("bnsh", [16, 1], F32, kind="ExternalInput")
    netb_d = nc.dram_tensor("netb", [2, 128], F32, kind="ExternalInput")
    lamb_d = nc.dram_tensor("lamb", [2, 128], F32, kind="ExternalInput")
    id_d = nc.dram_tensor("ident", [128, 128], BF16, kind="ExternalInput")
    out_d = nc.dram_tensor("out", [T, 256, 3136], F32, kind="ExternalOutput")

    with tile.TileContext(nc) as tc:
        with (
            tc.tile_pool(name="const", bufs=1) as cpool,
            tc.tile_pool(name="xp", bufs=1) as xpool,
            tc.tile_pool(name="big", bufs=1) as bigpool,
            tc.tile_pool(name="work", bufs=3) as wpool,
            tc.tile_pool(name="small", bufs=1) as spool,
            tc.tile_pool(name="cpsum", bufs=6, space="PSUM") as cpsum,
            tc.tile_pool(name="spsum", bufs=2, space="PSUM") as spsum,
        ):
            # ---- weights/constants; co_t=0 weight block first so PE can start
            wt_sb = cpool.tile([128, 36, 128], BF16)
            nc.sync.dma_start(out=wt_sb[:, 0:18], in_=wt_d.ap()[:, 0:18])
            # all 8 frames resident; frame0 split row-wise so the first conv
            # tiles (rows 0-9) unblock as early as possible
            xin = [xpool.tile([128, 2, PADSZ], BF16, tag=f"xin{f}",
                              name=f"xin{f}") for f in range(T)]
            for r0, r1 in ((0, 16), (16, 32), (32, 44), (44, 58)):
                sl0 = slice(r0 * HP, r1 * HP)
                for ci in range(2):
                    nc.sync.dma_start(out=xin[0][:, ci, sl0],
                                      in_=x_d.ap()[0, ci][:, sl0])
            nc.sync.dma_start(out=wt_sb[:, 18:36], in_=wt_d.ap()[:, 18:36])
            for f in range(1, T):
                nc.sync.dma_start(
                    out=xin[f][:], in_=x_d.ap()[f].rearrange("t p m -> p t m"))
            lamw_sb = cpool.tile([128, 4, 128], BF16)
            nc.sync.dma_start(out=lamw_sb[:], in_=lamw_d.ap().rearrange("w p m -> p w m"))
            w1t_sb = cpool.tile([8, 16], BF16)
            nc.sync.dma_start(out=w1t_sb[:], in_=w1t_d.ap())
            w2t_sb = cpool.tile([16, 3], BF16)
            nc.sync.dma_start(out=w2t_sb[:], in_=w2t_d.ap())
            bns_sb = cpool.tile([16, 1], F32)
            nc.sync.dma_start(out=bns_sb[:], in_=bns_d.ap())
            bnsh_sb = cpool.tile([16, 1], F32)
            nc.sync.dma_start(out=bnsh_sb[:], in_=bnsh_d.ap())
            netb_sb = cpool.tile([128, 2], F32)
            nc.sync.dma_start(out=netb_sb[:], in_=netb_d.ap().rearrange("t p -> p t"))
            lamb_sb = cpool.tile([128, 2], F32)
            nc.sync.dma_start(out=lamb_sb[:], in_=lamb_d.ap().rearrange("t p -> p t"))
            id_sb = cpool.tile([128, 128], BF16)
            nc.sync.dma_start(out=id_sb[:], in_=id_d.ap())

            # ---------------- prepass: strip sums -> T[ci, dy, dx, f] --------
            # T[tap] = S - r(dy) - c(dx) + corner; exact pooled conv sums are
            # then sum_tap w*T via tiny matmuls against the same wt tiles.
            T_all = spool.tile([128, 2, 9, T], F32)   # taps dy*3+dx
            ST = spool.tile([128, 2, 4, T], F32)      # r55, r0, c55, c0
            S_sb = spool.tile([128, T, 2], F32)
            zeros9 = spool.tile([128, 3, 3, 1], F32)
            nc.vector.memset(zeros9[:], 0.0)

            def strip_reduce(dst, src):
                nc.vector.tensor_reduce(out=dst, in_=src,
                                        axis=mybir.AxisListType.X, op=OP.add)

            for f in range(T):
                # full padded-frame sums, both ci at once (DVE keeps ACT free
                # for the conv PSUM copies)
                strip_reduce(S_sb[:, f], xin[f][:])
                for ci in range(2):
                    xf = xin[f][:, ci]
                    # row 55 (padded row 56), row 0 (padded row 1)
                    strip_reduce(ST[:, ci, 0, f:f + 1],
                                 xf[:, 56 * HP:57 * HP].rearrange("p (a c) -> p a c", a=1))
                    strip_reduce(ST[:, ci, 1, f:f + 1],
                                 xf[:, 1 * HP:2 * HP].rearrange("p (a c) -> p a c", a=1))
                    # col 55 (padded col 56), col 0 (padded col 1)
                    xcols = xf.rearrange("p (r c) -> p c r", c=HP)
                    strip_reduce(ST[:, ci, 2, f:f + 1], xcols[:, 56:57, :])
                    strip_reduce(ST[:, ci, 3, f:f + 1], xcols[:, 1:2, :])
                    Tv = T_all[:, ci].rearrange("p (dy dx) f -> p dy dx f", dx=3)
                    # init all taps to S
                    nc.vector.tensor_scalar_add(
                        out=Tv[:, :, :, f:f + 1], in0=zeros9[:],
                        scalar1=S_sb[:, f, ci:ci + 1])
                    # dy=0 excludes row55; dy=2 excludes row0
                    nc.vector.tensor_scalar_sub(
                        out=Tv[:, 0:1, :, f:f + 1], in0=Tv[:, 0:1, :, f:f + 1],
                        scalar1=ST[:, ci, 0, f:f + 1])
                    nc.vector.tensor_scalar_sub(
                        out=Tv[:, 2:3, :, f:f + 1], in0=Tv[:, 2:3, :, f:f + 1],
                        scalar1=ST[:, ci, 1, f:f + 1])
                    # dx=0 excludes col55; dx=2 excludes col0
                    nc.vector.tensor_scalar_sub(
                        out=Tv[:, :, 0:1, f:f + 1], in0=Tv[:, :, 0:1, f:f + 1],
                        scalar1=ST[:, ci, 2, f:f + 1])
                    nc.vector.tensor_scalar_sub(
                        out=Tv[:, :, 2:3, f:f + 1], in0=Tv[:, :, 2:3, f:f + 1],
                        scalar1=ST[:, ci, 3, f:f + 1])
                    # corners (doubly-excluded element added back)
                    for (dy, dx, off) in ((0, 0, 56 * HP + 56), (0, 2, 56 * HP + 1),
                                          (2, 0, 1 * HP + 56), (2, 2, 1 * HP + 1)):
                        nc.vector.tensor_add(
                            out=Tv[:, dy, dx, f:f + 1], in0=Tv[:, dy, dx, f:f + 1],
                            in1=xf[:, off:off + 1])
            Tb = spool.tile([128, 2, 9, T], BF16)
            nc.vector.tensor_copy(out=Tb[:], in_=T_all[:])

            # ---------------- conv state ----------------
            # ring of 4 frame outputs (phase-2 for frame p needs p-1, p, p+1)
            oraw = [bigpool.tile([128, 2, 3136], BF16, tag="oraw", name="oraw",
                                 bufs=4) for _ in range(T)]

            def conv_frame(f, ilv=None, gst=None):
                for co_t in range(2):
                    for sp in range(NSP):
                        y0 = sp * 8
                        ct = cpsum.tile([128, TW], F32, tag="conv", name="ct")
                        idx = 0
                        for ci_t in range(2):
                            xv = xin[f][:, ci_t].rearrange("p (r c) -> p r c", c=HP)
                            for dy in range(3):
                                for dx in range(3):
                                    w = co_t * 18 + ci_t * 9 + dy * 3 + dx
                                    nc.tensor.matmul(
                                        ct[:, :],
                                        wt_sb[:, w],
                                        xv[:, y0 + dy:y0 + dy + 8, dx:dx + 56],
                                        start=(idx == 0), stop=(idx == 17))
                                    idx += 1
                        nc.scalar.activation(
                            out=oraw[f][:, co_t, sp * TW:(sp + 1) * TW],
                            in_=ct[:, :], func=AF.Copy)
                        if ilv and (co_t, sp) in ilv:
                            for (p, ct2, chi) in ilv[(co_t, sp)]:
                                phase2_chunk(p, gst, ct2, chi)

            # ---------------- small ops (gates) ----------------
            def small_ops_a():
                # pooled_sum[co, f] = sum_taps w . T  (exact pooled conv sums)
                pooled_ps = spsum.tile([128, 2, T], F32, tag="sp", name="pooled_ps")
                for co_t in range(2):
                    idx = 0
                    for ci_t in range(2):
                        for tap in range(9):
                            w = co_t * 18 + ci_t * 9 + tap
                            nc.tensor.matmul(
                                pooled_ps[:, co_t], wt_sb[:, w],
                                Tb[:, ci_t, tap, :],
                                start=(idx == 0), stop=(idx == 17))
                            idx += 1
                pooled_sum = spool.tile([128, 2, T], F32)
                nc.vector.tensor_copy(out=pooled_sum[:], in_=pooled_ps[:])
                total = spool.tile([128, 2], F32)
                nc.vector.tensor_reduce(
                    out=total[:], in_=pooled_sum[:], axis=mybir.AxisListType.X,
                    op=OP.add)
                xgpre = spool.tile([128, 2], BF16)
                for t in range(2):
                    nc.vector.tensor_scalar(
                        out=xgpre[:, t:t + 1], in0=total[:, t:t + 1],
                        scalar1=1.0 / (T * 3136.0), scalar2=netb_sb[:, t:t + 1],
                        op0=OP.mult, op1=OP.add)
                xg_ps = spsum.tile([128, 2], F32, tag="sp", name="xg_ps")
                for ct_ in range(2):
                    for kt in range(2):
                        nc.tensor.matmul(
                            xg_ps[:, ct_:ct_ + 1], lamw_sb[:, kt * 2 + ct_],
                            xgpre[:, kt:kt + 1], start=(kt == 0), stop=(kt == 1))
                xg = spool.tile([128, 2], F32)
                for t in range(2):
                    nc.scalar.activation(
                        out=xg[:, t:t + 1], in_=xg_ps[:, t:t + 1], func=AF.Identity,
                        bias=lamb_sb[:, t:t + 1])
                bxg = spool.tile([128, 2], F32)
                nc.vector.tensor_add(out=bxg[:], in0=netb_sb[:], in1=xg[:])
                pooled = spool.tile([128, 2, T], F32)
                for t in range(2):
                    nc.vector.tensor_scalar(
                        out=pooled[:, t], in0=pooled_sum[:, t],
                        scalar1=1.0 / 3136.0, scalar2=bxg[:, t:t + 1],
                        op0=OP.mult, op1=OP.add)
                pooled_bf = spool.tile([128, 2, T], BF16)
                nc.vector.tensor_copy(out=pooled_bf[:], in_=pooled[:])
                pT_ps = spsum.tile([8, 256], BF16, tag="sp", name="pT_ps")
                for t in range(2):
                    nc.tensor.transpose(
                        pT_ps[:, t * 128:(t + 1) * 128], pooled_bf[:, t], id_sb[:])
                return pooled_ps, pooled, pooled_bf, pT_ps, bxg

            def small_ops_b(state):
                pooled_ps, pooled, pooled_bf, pT_ps, bxg = state
                pooledT = spool.tile([8, 256], BF16)
                nc.vector.tensor_copy(out=pooledT[:], in_=pT_ps[:])
                hdn_ps = spsum.tile([16, 256], F32, tag="sp", name="hdn_ps")
                nc.tensor.matmul(hdn_ps[:], w1t_sb[:], pooledT[:], start=True,
                                 stop=True)
                hdnr = spool.tile([16, 256], BF16)
                nc.scalar.activation(
                    out=hdnr[:], in_=hdn_ps[:], func=AF.Relu,
                    scale=bns_sb[:, 0:1], bias=bnsh_sb[:, 0:1])
                lgT_ps = spsum.tile([3, 256], F32, tag="sp", name="lgT_ps")
                nc.tensor.matmul(lgT_ps[:], w2t_sb[:], hdnr[:], start=True,
                                 stop=True)
                lgT = spool.tile([3, 256], BF16)
                nc.vector.tensor_copy(out=lgT[:], in_=lgT_ps[:])
                ew = spool.tile([128, 2, 3], F32)
                for t in range(2):
                    lg_ps = spsum.tile([128, 3], BF16, tag="sp", name="lg_ps")
                    nc.tensor.transpose(
                        lg_ps[:], lgT[:, t * 128:(t + 1) * 128], id_sb[0:3, 0:3])
                    nc.scalar.activation(out=ew[:, t], in_=lg_ps[:], func=AF.Exp)
                return pooled, bxg, ew

            def small_ops_c(state):
                pooled, bxg, ew = state
                es = spool.tile([128, 2], F32)
                nc.vector.tensor_reduce(
                    out=es[:], in_=ew[:], axis=mybir.AxisListType.X, op=OP.add)
                esr = spool.tile([128, 2], F32)
                nc.vector.reciprocal(out=esr[:], in_=es[:])
                wgt = spool.tile([128, 2, 3], F32)
                for t in range(2):
                    nc.vector.tensor_scalar_mul(
                        out=wgt[:, t], in0=ew[:, t], scalar1=esr[:, t:t + 1])
                # m[c,f] = mean_hw(lam_out) = temporal conv of pooled with wgt
                m = spool.tile([128, 2, T], F32)
                for t in range(2):
                    nc.vector.tensor_scalar_mul(
                        out=m[:, t], in0=pooled[:, t], scalar1=wgt[:, t, 1:2])
                    nc.vector.scalar_tensor_tensor(
                        out=m[:, t, 1:T], in0=pooled[:, t, 0:T - 1],
                        scalar=wgt[:, t, 0:1], in1=m[:, t, 1:T],
                        op0=OP.mult, op1=OP.add)
                    nc.vector.scalar_tensor_tensor(
                        out=m[:, t, 0:T - 1], in0=pooled[:, t, 1:T],
                        scalar=wgt[:, t, 2:3], in1=m[:, t, 0:T - 1],
                        op0=OP.mult, op1=OP.add)
                y = spool.tile([128, 2, T], F32)
                nc.vector.memset(y[:], 0.0)
                for t in range(2):
                    nc.vector.tensor_sub(
                        out=y[:, t, 0:T - 1], in0=m[:, t, 1:T], in1=m[:, t, 0:T - 1])
                y_bf = spool.tile([128, 2, T], BF16)
                nc.vector.tensor_copy(out=y_bf[:], in_=y[:])
                yT_ps = spsum.tile([8, 256], BF16, tag="sp", name="yT_ps")
                for t in range(2):
                    nc.tensor.transpose(
                        yT_ps[:, t * 128:(t + 1) * 128], y_bf[:, t], id_sb[:])
                return wgt, bxg, yT_ps

            def small_ops_d(state):
                wgt, bxg, yT_ps = state
                yT = spool.tile([8, 256], F32)
                nc.vector.tensor_copy(out=yT[:], in_=yT_ps[:])
                ycT = spool.tile([8, 256], F32)
                nc.vector.tensor_scalar_mul(out=ycT[:], in0=yT[:], scalar1=float(me[1]))
                nc.vector.scalar_tensor_tensor(
                    out=ycT[:, 1:256], in0=yT[:, 0:255], scalar=float(me[0]),
                    in1=ycT[:, 1:256], op0=OP.mult, op1=OP.add)
                nc.vector.scalar_tensor_tensor(
                    out=ycT[:, 0:255], in0=yT[:, 1:256], scalar=float(me[2]),
                    in1=ycT[:, 0:255], op0=OP.mult, op1=OP.add)
                gateT = spool.tile([8, 256], BF16)
                nc.scalar.activation(out=gateT[:], in_=ycT[:], func=AF.Sigmoid)
                gate_c = spool.tile([128, 2, T], F32)
                for t in range(2):
                    g_ps = spsum.tile([128, 8], BF16, tag="sp", name="g_ps")
                    nc.tensor.transpose(
                        g_ps[:], gateT[:, t * 128:(t + 1) * 128], id_sb[0:8, 0:8])
                    nc.vector.tensor_copy(out=gate_c[:, t], in_=g_ps[:])
                # per-(c,f) scalars for phase 2
                g0 = spool.tile([128, 2, T], F32)
                g1 = spool.tile([128, 2, T], F32)
                g2 = spool.tile([128, 2, T], F32)
                g = [g0, g1, g2]
                for k in range(3):
                    for t in range(2):
                        nc.vector.tensor_scalar_mul(
                            out=g[k][:, t], in0=gate_c[:, t], scalar1=wgt[:, t, k:k + 1])
                goffs = spool.tile([128, 2, T], F32)
                w01 = spool.tile([128, 2], F32)
                w12 = spool.tile([128, 2], F32)
                for t in range(2):
                    nc.vector.tensor_scalar_mul(
                        out=goffs[:, t], in0=gate_c[:, t], scalar1=bxg[:, t:t + 1])
                    nc.vector.tensor_add(
                        out=w12[:, t:t + 1], in0=wgt[:, t, 1:2], in1=wgt[:, t, 2:3])
                    nc.vector.tensor_add(
                        out=w01[:, t:t + 1], in0=wgt[:, t, 0:1], in1=wgt[:, t, 1:2])
                    nc.vector.tensor_mul(
                        out=goffs[:, t, 0:1], in0=goffs[:, t, 0:1], in1=w12[:, t:t + 1])
                    nc.vector.tensor_mul(
                        out=goffs[:, t, 7:8], in0=goffs[:, t, 7:8], in1=w01[:, t:t + 1])
                return g0, g1, g2, goffs

            # ---------------- phase 2: temporal conv + gate + store ----------
            CH = [(0, 896), (896, 1792), (1792, 2688), (2688, 3136)]

            def phase2_chunk(p, gst, co_t, chi):
                g0, g1, g2, goffs = gst
                g = [g0, g1, g2]
                c0, c1 = CH[chi]
                sl = slice(c0, c1)
                W2 = c1 - c0

                def o(ff):
                    return oraw[ff][:, co_t, sl]
                fin = wpool.tile([128, 896], F32, tag="fin", name="fin", bufs=3)
                if p == 0 or p == T - 1:
                    fa, ka, fb, kb = (0, 1, 1, 2) if p == 0 else (T - 2, 0, T - 1, 1)
                    nc.vector.tensor_scalar(
                        out=fin[:, :W2], in0=o(fa),
                        scalar1=g[ka][:, co_t, p:p + 1],
                        scalar2=goffs[:, co_t, p:p + 1],
                        op0=OP.mult, op1=OP.add)
                    nc.vector.scalar_tensor_tensor(
                        out=fin[:, :W2], in0=o(fb),
                        scalar=g[kb][:, co_t, p:p + 1],
                        in1=fin[:, :W2], op0=OP.mult, op1=OP.add)
                else:
                    nc.vector.tensor_scalar(
                        out=fin[:, :W2], in0=o(p - 1),
                        scalar1=g0[:, co_t, p:p + 1],
                        scalar2=goffs[:, co_t, p:p + 1],
                        op0=OP.mult, op1=OP.add)
                    nc.vector.scalar_tensor_tensor(
                        out=fin[:, :W2], in0=o(p),
                        scalar=g1[:, co_t, p:p + 1],
                        in1=fin[:, :W2], op0=OP.mult, op1=OP.add)
                    nc.vector.scalar_tensor_tensor(
                        out=fin[:, :W2], in0=o(p + 1),
                        scalar=g2[:, co_t, p:p + 1],
                        in1=fin[:, :W2], op0=OP.mult, op1=OP.add)
                eng = nc.sync if (chi % 2 == 0) else nc.gpsimd
                eng.dma_start(
                    out=out_d.ap()[p, co_t * 128:(co_t + 1) * 128, sl],
                    in_=fin[:, :W2])

            def phase2_frame(p, gst):
                for co_t in range(2):
                    for chi in range(4):
                        phase2_chunk(p, gst, co_t, chi)

            # ---------------- schedule ----------------
            conv_frame(0)
            conv_frame(1)
            st_a = small_ops_a()
            conv_frame(2)
            st_b = small_ops_b(st_a)
            st_c = small_ops_c(st_b)
            conv_frame(3)
            gst = small_ops_d(st_c)
            phase2_frame(0, gst)
            phase2_frame(1, gst)
            phase2_frame(2, gst)
            for f in range(4, T - 1):
                conv_frame(f)
                phase2_frame(f - 1, gst)
            # last conv frame: interleave phase-2 of frames 6 and 7 at chunk
            # granularity right behind the tiles they depend on (kills the tail)
            ilv = {(0, 1): [(6, 0, 0), (7, 0, 0)], (0, 3): [(6, 0, 1), (7, 0, 1)],
                   (0, 5): [(6, 0, 2), (7, 0, 2)], (0, 6): [(6, 0, 3), (7, 0, 3)],
                   (1, 1): [(6, 1, 0), (7, 1, 0)], (1, 3): [(6, 1, 1), (7, 1, 1)],
                   (1, 5): [(6, 1, 2), (7, 1, 2)], (1, 6): [(6, 1, 3), (7, 1, 3)]}
            conv_frame(T - 1, ilv, gst)

    nc.compile()
    return nc


def _prep(inputs):
    x = np.asarray(inputs["x"], np.float32)          # (64,256,56,56)
    net_w = np.asarray(inputs["net_w"], np.float32)  # (256,256,3,3)
    net_b = np.asarray(inputs["net_b"], np.float32)
    lam_w = np.asarray(inputs["lam_w"], np.float32)
    lam_b = np.asarray(inputs["lam_b"], np.float32)
    mlp_w1 = np.asarray(inputs["mlp_w1"], np.float32)  # (16,8)
    mlp_w2 = np.asarray(inputs["mlp_w2"], np.float32)  # (3,16)
    bn_g = np.asarray(inputs["bn_gamma"], np.float32)
    bn_b = np.asarray(inputs["bn_beta"], np.float32)
    bn_m = np.asarray(inputs["bn_mean"], np.float32)
    bn_v = np.asarray(inputs["bn_var"], np.float32)
    me_w = np.asarray(inputs["me_w"], np.float32)

    bf = ml_dtypes.bfloat16
    xs = x.reshape(NCORES, T, 2, 128, 56, 56)
    xpad = np.zeros((NCORES, T, 2, 128, HP, HP), dtype=bf)
    xpad[:, :, :, :, 1:57, 1:57] = xs.astype(bf)
    xpad = np.ascontiguousarray(xpad.reshape(NCORES, T, 2, 128, PADSZ))

    # wt[p=ci, w_idx, m=co] with w_idx = co_t*18 + ci_t*9 + dy*3 + dx
    wtb = net_w.reshape(2, 128, 2, 128, 3, 3)          # co_t co ci_t ci dy dx
    wt = wtb.transpose(3, 0, 2, 4, 5, 1)               # ci co_t ci_t dy dx co
    wt = np.ascontiguousarray(wt.reshape(128, 36, 128).astype(bf))
    lamw = lam_w.T.reshape(2, 128, 2, 128).transpose(0, 2, 1, 3)
    lamw = np.ascontiguousarray(lamw.reshape(4, 128, 128).astype(bf))
    w1t = np.ascontiguousarray(mlp_w1.T.astype(bf))      # (8,16)
    w2t = np.ascontiguousarray(mlp_w2.T.astype(bf))      # (16,3)
    bns = (bn_g / np.sqrt(bn_v + 1e-5)).astype(np.float32).reshape(16, 1)
    bnsh = (bn_b - bn_m * bns[:, 0]).astype(np.float32).reshape(16, 1)
    netb = np.ascontiguousarray(net_b.reshape(2, 128))
    lamb = np.ascontiguousarray(lam_b.reshape(2, 128))
    ident = np.eye(128, dtype=bf)

    common = dict(wt=wt, lamw=lamw, w1t=w1t, w2t=w2t, bns=bns, bnsh=bnsh,
                  netb=netb, lamb=lamb, ident=ident)
    in_maps = [dict(x=xpad[i], **common) for i in range(NCORES)]
    return in_maps, tuple(float(v) for v in me_w)


def kernel(**inputs):
    in_maps, me = _prep(inputs)
    nc = _CACHE.get(me)
    if nc is None:
        nc = _build(me)
        _CACHE[me] = nc
    res = run_bass_kernel_spmd(nc, in_maps, core_ids=list(range(NCORES)))
    out = np.stack([res.results[i]["out"] for i in range(NCORES)])  # (8,8,256,3136)
    return np.ascontiguousarray(out.reshape(64, 256, 56, 56))
